# revision 7
# baseline (speedup 1.0000x reference)
"""Trainium2 Bass kernel for AdaptiveWindowLoss (segment_reduce).

Reference semantics (per row b of scores[B,S], labels[B,S]):
    k      = ceil(1 / max(density_b, 0.1))            # k in [1, 10]
    win    = { t : |t - t_star_b| <= k }              # <= 21 columns
    w      = exp(-|t - t_star_b|) * win ; w /= sum(w)
    ref_avg= sum(scores*w*[lab==1 & win]) / max(cnt1, 1)
    dev_avg= sum(scores*w*[lab==0 & win]) / max(cnt0, 1)
    valid  = cnt1>0 and cnt0>0
    loss_b = softplus(-(ref_avg - dev_avg))
    out    = sum(loss_b * valid) / max(n_valid, 1)    (0 if n_valid == 0)

Only the <=21-wide window around t_star matters, so each core gathers a
fixed 21-wide slab per row with one indirect DMA (per-row start =
clamp(t_star-10, 0, S-21)) instead of reading the full 4096 columns.

Distribution: batch rows sharded 1024/core across 8 cores. Each core
emits [sum(loss*valid), sum(valid)]; the host adds the 8 pairs and does
the final division (16 floats - cheaper than a device AllReduce).
"""

import sys

for _p in ("/opt/trn_rl_repo", "/root/.axon_site/_ro/trn_rl_repo"):
    if _p not in sys.path:
        sys.path.append(_p)

import numpy as np

from concourse import bass, bacc, mybir
import concourse.tile as tile
from concourse.bass_utils import run_bass_kernel_spmd

B, S = 8192, 4096
NCORES = 8
BL = B // NCORES        # 1024 rows per core
P = 128                 # SBUF partitions
J = BL // P             # 8 windows per partition
KMAX = 10               # k = ceil(1/max(d,0.1)) <= 10
W = 2 * KMAX + 1        # 21-wide gather covers every possible window
F32 = mybir.dt.float32
I32 = mybir.dt.int32

_graph_cache = None


def _build_graph():
    nc = bacc.Bacc()
    sc_ext = nc.declare_dram_parameter("scores", [BL * S], F32, isOutput=False)
    lb_ext = nc.declare_dram_parameter("labels", [BL * S], F32, isOutput=False)
    de_ext = nc.declare_dram_parameter("dens", [P, J], F32, isOutput=False)
    ts_ext = nc.declare_dram_parameter("tstar", [P, J], I32, isOutput=False)
    out_ext = nc.declare_dram_parameter("out", [2, 1], F32, isOutput=True)

    TT = mybir.AluOpType
    with tile.TileContext(nc) as tc:
        with (
            tc.tile_pool(name="sbuf", bufs=1) as pool,
            tc.tile_pool(name="psum", bufs=1, space="PSUM") as psum,
        ):
            # const tiles ([P,1], broadcast along free dim where needed) so
            # every elementwise op lowers to TensorTensor: the TensorScalarPtr
            # encoding only has one sync-wait slot and cross-engine joins
            # need two.
            def const(v):
                t = pool.tile([P, 1], F32, tag=f"const_{v}")
                nc.vector.memset(t[:], float(v))
                return t

            c_zero = const(0.0)
            c_one = const(1.0)
            c_k = const(float(KMAX))
            c_smw = const(float(S - W))
            c_p1 = const(0.1)
            c_big = const(60.0)

            def tt(out, in0, in1, op):
                nc.vector.tensor_tensor(out=out, in0=in0, in1=in1, op=op)

            def cb(c, shape):
                return c[:].to_broadcast(shape)

            # ---- per-row parameters: local row r = p*J + j lives at [p, j]
            ts_i = pool.tile([P, J], I32)
            dn = pool.tile([P, J], F32)
            nc.sync.dma_start(out=ts_i[:], in_=ts_ext[:])
            nc.sync.dma_start(out=dn[:], in_=de_ext[:])

            tf = pool.tile([P, J], F32)
            nc.vector.tensor_copy(out=tf[:], in_=ts_i[:])

            # window start = clamp(t_star - 10, 0, S - W); exact in f32
            st = pool.tile([P, J], F32)
            tt(st[:], tf[:], cb(c_k, [P, J]), TT.subtract)
            tt(st[:], st[:], cb(c_zero, [P, J]), TT.max)
            tt(st[:], st[:], cb(c_smw, [P, J]), TT.min)
            # a = in-window position of t_star
            a = pool.tile([P, J], F32)
            tt(a[:], tf[:], st[:], TT.subtract)
            # d' = max(density, 0.1); rhs = 1 + d'
            dp = pool.tile([P, J], F32)
            tt(dp[:], dn[:], cb(c_p1, [P, J]), TT.max)
            rhs = pool.tile([P, J], F32)
            tt(rhs[:], dp[:], cb(c_one, [P, J]), TT.add)

            # gather element index = r*S + start  (int32)
            rb = pool.tile([P, J], I32)
            nc.gpsimd.iota(rb[:], pattern=[[S, J]], base=0, channel_multiplier=J * S)
            st_i = pool.tile([P, J], I32)
            nc.vector.tensor_copy(out=st_i[:], in_=st[:])
            idx = pool.tile([P, J], I32)
            tt(idx[:], rb[:], st_i[:], TT.add)

            # ---- indirect gathers: window slab [P, J*W]
            scw = pool.tile([P, J * W], F32)
            lbw = pool.tile([P, J * W], F32)
            # source viewed [BL*S, 1] so the per-index element offset is idx*1
            nc.gpsimd.indirect_dma_start(
                out=scw[:], out_offset=None,
                in_=sc_ext[:].rearrange("(a b) -> a b", b=1),
                in_offset=bass.IndirectOffsetOnAxis(ap=idx[:], axis=0),
            )
            nc.gpsimd.indirect_dma_start(
                out=lbw[:], out_offset=None,
                in_=lb_ext[:].rearrange("(a b) -> a b", b=1),
                in_offset=bass.IndirectOffsetOnAxis(ap=idx[:], axis=0),
            )

            def b3(ap2):  # [P, J] -> broadcast [P, J, W]
                return ap2.to_broadcast([P, J, W])

            def v3(ap2):  # [P, J*W] -> [P, J, W] view
                return ap2.rearrange("p (j w) -> p j w", w=W)

            JW = J * W
            # ---- window-level compute on [P, J*W]
            # t within window: 0..W-1 repeated J times (values exact in f32)
            tor = pool.tile([P, JW], F32)
            nc.gpsimd.iota(
                tor[:], pattern=[[0, J], [1, W]], base=0, channel_multiplier=0,
                allow_small_or_imprecise_dtypes=True,
            )
            # dist = |t - a| = max(t-a, a-t)
            d1 = pool.tile([P, JW], F32)
            d2 = pool.tile([P, JW], F32)
            tt(v3(d1[:]), v3(tor[:]), b3(a[:]), TT.subtract)
            tt(v3(d2[:]), b3(a[:]), v3(tor[:]), TT.subtract)
            dist = pool.tile([P, JW], F32)
            tt(dist[:], d1[:], d2[:], TT.max)
            # window mask: dist <= ceil(1/d')  <=>  dist*d' < 1+d'
            lhs = pool.tile([P, JW], F32)
            tt(v3(lhs[:]), v3(dist[:]), b3(dp[:]), TT.mult)
            wm = pool.tile([P, JW], F32)
            tt(v3(wm[:]), v3(lhs[:]), b3(rhs[:]), TT.is_lt)
            # masked weights in one Exp: arg = 60*wm - dist - 60
            #   inside window: -dist (exact); outside: <= -60 -> exp ~ 0
            t1 = pool.tile([P, JW], F32)
            tt(v3(t1[:]), v3(wm[:]), cb(c_big, [P, J, W]), TT.mult)
            t2 = pool.tile([P, JW], F32)
            tt(t2[:], t1[:], dist[:], TT.subtract)
            arg = pool.tile([P, JW], F32)
            tt(v3(arg[:]), v3(t2[:]), cb(c_big, [P, J, W]), TT.subtract)
            wn = pool.tile([P, JW], F32)
            nc.scalar.activation(
                out=wn[:], in_=arg[:], func=mybir.ActivationFunctionType.Exp
            )
            # label masks
            l1 = pool.tile([P, JW], F32)
            l0 = pool.tile([P, JW], F32)
            tt(v3(l1[:]), v3(lbw[:]), cb(c_one, [P, J, W]), TT.is_equal)
            tt(v3(l0[:]), v3(lbw[:]), cb(c_zero, [P, J, W]), TT.is_equal)
            m1 = pool.tile([P, JW], F32)
            m0 = pool.tile([P, JW], F32)
            tt(m1[:], wm[:], l1[:], TT.mult)
            tt(m0[:], wm[:], l0[:], TT.mult)
            sw = pool.tile([P, JW], F32)
            tt(sw[:], scw[:], wn[:], TT.mult)
            sw1 = pool.tile([P, JW], F32)
            sw0 = pool.tile([P, JW], F32)
            tt(sw1[:], sw[:], l1[:], TT.mult)
            tt(sw0[:], sw[:], l0[:], TT.mult)

            # ---- per-window reductions [P, J, W] -> [P, J]
            def red(src):
                dst = pool.tile([P, J], F32)
                nc.vector.tensor_reduce(
                    out=dst[:], in_=v3(src[:]), axis=mybir.AxisListType.X,
                    op=mybir.AluOpType.add,
                )
                return dst

            sum_w = red(wn)
            c1 = red(m1)
            c0 = red(m0)
            s1 = red(sw1)
            s0 = red(sw0)

            # ---- row-level math on [P, J]
            u1 = pool.tile([P, J], F32)
            u0 = pool.tile([P, J], F32)
            tt(u1[:], c1[:], cb(c_one, [P, J]), TT.max)
            tt(u0[:], c0[:], cb(c_one, [P, J]), TT.max)
            inv_w = pool.tile([P, J], F32)
            inv1 = pool.tile([P, J], F32)
            inv0 = pool.tile([P, J], F32)
            nc.vector.reciprocal(out=inv_w[:], in_=sum_w[:])
            nc.vector.reciprocal(out=inv1[:], in_=u1[:])
            nc.vector.reciprocal(out=inv0[:], in_=u0[:])
            ra = pool.tile([P, J], F32)
            rd = pool.tile([P, J], F32)
            tt(ra[:], s1[:], inv1[:], TT.mult)
            tt(rd[:], s0[:], inv0[:], TT.mult)
            delta = pool.tile([P, J], F32)
            tt(delta[:], ra[:], rd[:], TT.subtract)
            tt(delta[:], delta[:], inv_w[:], TT.mult)
            # valid = (c1 > 0) & (c0 > 0)
            v1 = pool.tile([P, J], F32)
            v0 = pool.tile([P, J], F32)
            tt(v1[:], c1[:], cb(c_zero, [P, J]), TT.is_gt)
            tt(v0[:], c0[:], cb(c_zero, [P, J]), TT.is_gt)
            val = pool.tile([P, J], F32)
            tt(val[:], v1[:], v0[:], TT.mult)
            # loss = softplus(-delta) = max(-delta,0) + log(1 + exp(-|delta|))
            nneg = pool.tile([P, J], F32)
            tt(nneg[:], cb(c_zero, [P, J]), delta[:], TT.subtract)
            mx = pool.tile([P, J], F32)
            tt(mx[:], nneg[:], cb(c_zero, [P, J]), TT.max)
            ad = pool.tile([P, J], F32)
            tt(ad[:], delta[:], nneg[:], TT.max)
            en = pool.tile([P, J], F32)
            nc.scalar.activation(
                out=en[:], in_=ad[:], func=mybir.ActivationFunctionType.Exp,
                scale=-1.0,
            )
            lg = pool.tile([P, J], F32)
            nc.scalar.activation(
                out=lg[:], in_=en[:], func=mybir.ActivationFunctionType.Ln,
                bias=1.0,
            )
            li = pool.tile([P, J], F32)
            tt(li[:], mx[:], lg[:], TT.add)
            lv = pool.tile([P, J], F32)
            tt(lv[:], li[:], val[:], TT.mult)

            # ---- reduce to per-core partials: [sum(loss*valid), sum(valid)]
            lsum = pool.tile([P, 1], F32)
            vsum = pool.tile([P, 1], F32)
            nc.vector.tensor_reduce(
                out=lsum[:], in_=lv[:], axis=mybir.AxisListType.X,
                op=mybir.AluOpType.add,
            )
            nc.vector.tensor_reduce(
                out=vsum[:], in_=val[:], axis=mybir.AxisListType.X,
                op=mybir.AluOpType.add,
            )
            s2 = pool.tile([P, 2], F32)
            nc.vector.tensor_copy(out=s2[:, 0:1], in_=lsum[:])
            nc.vector.tensor_copy(out=s2[:, 1:2], in_=vsum[:])
            ones = pool.tile([P, 1], F32)
            nc.vector.memset(ones[:], 1.0)
            ps = psum.tile([2, 1], F32)
            nc.tensor.matmul(
                out=ps[:], lhsT=s2[:], rhs=ones[:], start=True, stop=True
            )
            res = pool.tile([2, 1], F32)
            nc.vector.tensor_copy(out=res[:], in_=ps[:])
            nc.sync.dma_start(out=out_ext[:], in_=res[:])

    return nc


def _make_in_maps(scores, labels, dens, tstar):
    in_maps = []
    for c in range(NCORES):
        r0, r1 = c * BL, (c + 1) * BL
        in_maps.append(
            {
                "scores": np.ascontiguousarray(scores[r0:r1]).reshape(-1),
                "labels": np.ascontiguousarray(labels[r0:r1]).reshape(-1),
                "dens": np.ascontiguousarray(dens[r0:r1]).reshape(P, J),
                "tstar": np.ascontiguousarray(tstar[r0:r1]).reshape(P, J),
            }
        )
    return in_maps


def _prep_inputs(inputs):
    scores = np.asarray(inputs["scores"], dtype=np.float32)
    labels = np.asarray(inputs["labels"], dtype=np.float32)
    dens = np.asarray(inputs["checkpoint_density"], dtype=np.float32)
    tstar = np.asarray(inputs["t_star"]).astype(np.int32)
    assert scores.shape == (B, S) and labels.shape == (B, S)
    return _make_in_maps(scores, labels, dens, tstar)


def _combine(per_core_outs):
    parts = np.stack(
        [np.asarray(o, dtype=np.float64).reshape(2) for o in per_core_outs]
    )
    total_loss, n_valid = parts.sum(axis=0)
    if n_valid <= 0:
        return np.zeros((), dtype=np.float32)
    return np.float32(total_loss / max(n_valid, 1.0)).reshape(())


def get_graph():
    global _graph_cache
    if _graph_cache is None:
        nc = _build_graph()
        # Bacc defers register allocation and multi-wait splitting (HW allows
        # one sync wait per compute instruction) to its compile pass, which
        # runs in finalize().
        nc.finalize()
        _graph_cache = nc
    return _graph_cache


def kernel(**inputs) -> np.ndarray:
    in_maps = _prep_inputs(inputs)
    nc = get_graph()
    res = run_bass_kernel_spmd(nc, in_maps, core_ids=list(range(NCORES))).results
    return _combine([res[i]["out"] for i in range(NCORES)])


# revision 8
# speedup vs baseline: 1.1073x; 1.1073x over previous
"""Trainium2 Bass kernel for AdaptiveWindowLoss (segment_reduce).

Reference semantics (per row b of scores[B,S], labels[B,S]):
    k      = ceil(1 / max(density_b, 0.1))            # k in [1, 10]
    win    = { t : |t - t_star_b| <= k }              # <= 21 columns
    w      = exp(-|t - t_star_b|) * win ; w /= sum(w)
    ref_avg= sum(scores*w*[lab==1 & win]) / max(cnt1, 1)
    dev_avg= sum(scores*w*[lab==0 & win]) / max(cnt0, 1)
    valid  = cnt1>0 and cnt0>0
    loss_b = softplus(-(ref_avg - dev_avg))
    out    = sum(loss_b * valid) / max(n_valid, 1)    (0 if n_valid == 0)

Only the <=21-wide window around t_star matters, so each core gathers a
fixed 21-wide slab per row with one indirect DMA per tensor (per-row
start = clamp(t_star-10, 0, S-21)) instead of reading the full 4096
columns (~0.2% of the naive memory traffic).

Distribution: batch rows sharded 1024/core across 8 cores. Each core
emits [sum(loss*valid), sum(valid)]; the host adds the 8 pairs and does
the final division (16 floats - cheaper than a device AllReduce).

Numerical notes:
 - window mask dist<=ceil(1/d') is evaluated as dist*d' < 1+d' (exact
   in real arithmetic for integer dist; float edge cases are measure-
   zero and bounded by the 2e-2 rel-err gate).
 - masked weights are built inside the Exp argument (60*wm - 60 - dist)
   so no DVE multiply is needed on the ACT output; outside-window
   contributions are <= e^-60.
 - label masks: (lab==0) sums are derived as win_total - (lab==1) sums
   since labels are exactly {0,1}.
"""

import sys

for _p in ("/opt/trn_rl_repo", "/root/.axon_site/_ro/trn_rl_repo"):
    if _p not in sys.path:
        sys.path.append(_p)

import numpy as np

from concourse import bass, bacc, mybir
import concourse.tile as tile
from concourse.bass_utils import run_bass_kernel_spmd
from concourse.hw_specs import get_activation_tables

B, S = 8192, 4096
NCORES = 8
BL = B // NCORES        # 1024 rows per core
P = 128                 # SBUF partitions
J = BL // P             # 8 windows per partition
KMAX = 10               # k = ceil(1/max(d,0.1)) <= 10
W = 2 * KMAX + 1        # 21-wide gather covers every possible window
JW = J * W
F32 = mybir.dt.float32
I32 = mybir.dt.int32

_graph_cache = None


def _preload_act_table(nc):
    """Pre-place one ACT function-table load that covers Exp+Ln so the
    compile pass doesn't insert a second mid-kernel table swap (~2.7us:
    table DMA + forced scalar-engine drain)."""
    tables = get_activation_tables(nc.m.arch)
    need = {
        mybir.ActivationFunctionType.Exp,
        mybir.ActivationFunctionType.Ln,
        mybir.ActivationFunctionType.Identity,
        mybir.ActivationFunctionType.Copy,
    }
    set_id = None
    for i, (_name, funcs) in enumerate(tables.items()):
        if need <= funcs:
            set_id = i
            break
    if set_id is None:
        return  # fall back to automatic placement
    inst = mybir.InstLoadActFuncSet(
        name=nc.get_next_instruction_name(),
        act_func_set_id=set_id,
        ins=[],
        outs=[],
    )
    inst.engine = mybir.EngineType.Activation
    nc.register_instruction(inst)
    entry = nc.main_func.blocks[0]
    pos = 0
    if nc.scalar.preamble_end is not None:
        try:
            pos = entry.instructions.index(nc.scalar.preamble_end) + 1
        except ValueError:
            pos = 0
    entry.instructions.insert(pos, inst)


def _build_graph():
    nc = bacc.Bacc()
    sc_ext = nc.declare_dram_parameter("scores", [BL * S], F32, isOutput=False)
    lb_ext = nc.declare_dram_parameter("labels", [BL * S], F32, isOutput=False)
    # packed per-row params: cols [0,J) = t_star int32, [J,2J) = density f32 bits
    me_ext = nc.declare_dram_parameter("meta", [P, 2 * J], I32, isOutput=False)
    out_ext = nc.declare_dram_parameter("out", [2, 1], F32, isOutput=True)

    TT = mybir.AluOpType
    AF = mybir.ActivationFunctionType

    with tile.TileContext(nc) as tc:
        with (
            tc.tile_pool(name="sbuf", bufs=1) as pool,
            tc.tile_pool(name="psum", bufs=1, space="PSUM") as psum,
        ):
            def tt(out, in0, in1, op):
                nc.vector.tensor_tensor(out=out, in0=in0, in1=in1, op=op)

            def ts(out, in0, s1, op0, s2=None, op1=None):
                kw = {}
                if op1 is not None:
                    kw = dict(scalar2=s2, op1=op1)
                else:
                    kw = dict(scalar2=None)
                nc.vector.tensor_scalar(out=out, in0=in0, scalar1=s1, op0=op0, **kw)

            def b3(ap2):  # [P, J] -> broadcast [P, J, W]
                return ap2.to_broadcast([P, J, W])

            def v3(ap2):  # [P, n*W] -> [P, n, W] view
                return ap2.rearrange("p (j w) -> p j w", w=W)

            # ---- load packed params with ONE gpsimd DMA; index math stays
            # on gpsimd so the gathers need no cross-engine hop.
            meta = pool.tile([P, 2 * J], I32)
            nc.gpsimd.dma_start(out=meta[:], in_=me_ext[:])
            ts_i = meta[:, 0:J]
            dn = meta[:, J : 2 * J].bitcast(F32)

            # rowbase iota: r = p*J + j  ->  r*S
            rb = pool.tile([P, J], I32)
            nc.gpsimd.iota(rb[:], pattern=[[S, J]], base=0, channel_multiplier=J * S)
            # t within window: 0..W-1 repeated J times (exact in f32)
            tor = pool.tile([P, JW], F32)
            nc.gpsimd.iota(
                tor[:], pattern=[[0, J], [1, W]], base=0, channel_multiplier=0,
                allow_small_or_imprecise_dtypes=True,
            )

            # start = clamp(t_star - 10, 0, S-W)   (int32, on gpsimd)
            st_i = pool.tile([P, J], I32)
            nc.gpsimd.tensor_scalar(
                out=st_i[:], in0=ts_i, scalar1=-KMAX, scalar2=0,
                op0=TT.add, op1=TT.max,
            )
            nc.gpsimd.tensor_scalar(
                out=st_i[:], in0=st_i[:], scalar1=S - W, scalar2=None, op0=TT.min
            )
            idx = pool.tile([P, J], I32)
            nc.gpsimd.tensor_tensor(out=idx[:], in0=rb[:], in1=st_i[:], op=TT.add)
            # a = t_star - start = in-window position of t_star
            a_i = pool.tile([P, J], I32)
            nc.gpsimd.tensor_tensor(out=a_i[:], in0=ts_i, in1=st_i[:], op=TT.subtract)

            # ---- indirect gathers: window slab [P, J*W]
            # source viewed [BL*S, 1] so the per-index element offset is idx*1
            scw = pool.tile([P, JW], F32)
            lbw = pool.tile([P, JW], F32)
            nc.gpsimd.indirect_dma_start(
                out=scw[:], out_offset=None,
                in_=sc_ext[:].rearrange("(a b) -> a b", b=1),
                in_offset=bass.IndirectOffsetOnAxis(ap=idx[:], axis=0),
            )
            nc.gpsimd.indirect_dma_start(
                out=lbw[:], out_offset=None,
                in_=lb_ext[:].rearrange("(a b) -> a b", b=1),
                in_offset=bass.IndirectOffsetOnAxis(ap=idx[:], axis=0),
            )

            # ---- overlapped with the gathers: per-row f32 prep on DVE
            a = pool.tile([P, J], F32)
            nc.vector.tensor_copy(out=a[:], in_=a_i[:])
            dp = pool.tile([P, J], F32)    # d' = max(density, 0.1)
            ts(dp[:], dn, 0.1, TT.max)
            rhs = pool.tile([P, J], F32)   # 1 + d'
            ts(rhs[:], dn, 0.1, TT.max, 1.0, TT.add)

            # dist = |t - a| = max(t-a, a-t)
            d1 = pool.tile([P, JW], F32)
            d2 = pool.tile([P, JW], F32)
            tt(v3(d1[:]), v3(tor[:]), b3(a[:]), TT.subtract)
            tt(v3(d2[:]), b3(a[:]), v3(tor[:]), TT.subtract)
            dist = pool.tile([P, JW], F32)
            tt(dist[:], d1[:], d2[:], TT.max)
            # window mask: dist <= ceil(1/d')  <=>  dist*d' < 1+d'
            lhs = pool.tile([P, JW], F32)
            tt(v3(lhs[:]), v3(dist[:]), b3(dp[:]), TT.mult)

            # big reduce tile: 5 sections of [P, JW]:
            #   0: wn (masked exp weights)   1: wm (window mask)
            #   2: m1 (wm & lab==1)          3: sw (scores * wn)
            #   4: sw1 (sw & lab==1)
            big = pool.tile([P, 5 * JW], F32)
            wm = big[:, 1 * JW : 2 * JW]
            tt(v3(wm), v3(lhs[:]), b3(rhs[:]), TT.is_lt)
            # exp argument: 60*wm - 60 - dist  (= -dist inside the window)
            t1 = pool.tile([P, JW], F32)
            ts(t1[:], wm, 60.0, TT.mult, -60.0, TT.add)
            arg = pool.tile([P, JW], F32)
            tt(arg[:], t1[:], dist[:], TT.subtract)
            wn = big[:, 0:JW]
            nc.scalar.activation(out=wn, in_=arg[:], func=AF.Exp)
            # label mask and products
            l1 = pool.tile([P, JW], F32)
            ts(l1[:], lbw[:], 1.0, TT.is_equal)
            tt(big[:, 2 * JW : 3 * JW], wm, l1[:], TT.mult)       # m1
            sw = big[:, 3 * JW : 4 * JW]
            tt(sw, scw[:], wn, TT.mult)                           # sw
            tt(big[:, 4 * JW : 5 * JW], sw, l1[:], TT.mult)       # sw1

            # ---- one reduce for all five sections: [P,5J,W] -> [P,5J]
            red = pool.tile([P, 5 * J], F32)
            nc.vector.tensor_reduce(
                out=red[:], in_=v3(big[:]), axis=mybir.AxisListType.X, op=TT.add
            )
            sum_w = red[:, 0:J]
            cw = red[:, J : 2 * J]          # c1 + c0 (exact)
            c1 = red[:, 2 * J : 3 * J]
            s_tot = red[:, 3 * J : 4 * J]   # s1 + s0 (+ ~1e-25 eps)
            s1 = red[:, 4 * J : 5 * J]

            c0 = pool.tile([P, J], F32)
            tt(c0[:], cw, c1, TT.subtract)
            s0 = pool.tile([P, J], F32)
            tt(s0[:], s_tot, s1, TT.subtract)

            # ---- row-level math on [P, J]
            u1 = pool.tile([P, J], F32)
            u0 = pool.tile([P, J], F32)
            ts(u1[:], c1, 1.0, TT.max)
            ts(u0[:], c0[:], 1.0, TT.max)
            inv_w = pool.tile([P, J], F32)
            inv1 = pool.tile([P, J], F32)
            inv0 = pool.tile([P, J], F32)
            nc.vector.reciprocal(out=inv_w[:], in_=sum_w)
            nc.vector.reciprocal(out=inv1[:], in_=u1[:])
            nc.vector.reciprocal(out=inv0[:], in_=u0[:])
            ra = pool.tile([P, J], F32)
            rd = pool.tile([P, J], F32)
            tt(ra[:], s1, inv1[:], TT.mult)
            tt(rd[:], s0[:], inv0[:], TT.mult)
            delta = pool.tile([P, J], F32)
            tt(delta[:], ra[:], rd[:], TT.subtract)
            tt(delta[:], delta[:], inv_w[:], TT.mult)

            # final reduce tile: [P, 0:J] = loss*valid, [P, J:2J] = valid
            sl2 = pool.tile([P, 2 * J], F32)
            val = sl2[:, J : 2 * J]
            vm = pool.tile([P, J], F32)
            tt(vm[:], c1, c0[:], TT.min)
            ts(val, vm[:], 0.0, TT.is_gt)    # valid = (min(c1,c0) > 0)

            # loss = softplus(-delta) = max(-delta,0) + log(1+exp(-|delta|))
            nneg = pool.tile([P, J], F32)
            ts(nneg[:], delta[:], -1.0, TT.mult)
            mx = pool.tile([P, J], F32)
            ts(mx[:], nneg[:], 0.0, TT.max)
            ad = pool.tile([P, J], F32)
            tt(ad[:], delta[:], nneg[:], TT.max)
            en = pool.tile([P, J], F32)
            nc.scalar.activation(out=en[:], in_=ad[:], func=AF.Exp, scale=-1.0)
            lg = pool.tile([P, J], F32)
            nc.scalar.activation(out=lg[:], in_=en[:], func=AF.Ln, bias=1.0)
            li = pool.tile([P, J], F32)
            tt(li[:], mx[:], lg[:], TT.add)
            tt(sl2[:, 0:J], li[:], val, TT.mult)

            # ---- [P,2,J] -> [P,2] -> matmul with ones -> [2,1] partials
            s2 = pool.tile([P, 2], F32)
            nc.vector.tensor_reduce(
                out=s2[:], in_=sl2[:].rearrange("p (g j) -> p g j", j=J),
                axis=mybir.AxisListType.X, op=TT.add,
            )
            ones = pool.tile([P, 1], F32)
            nc.vector.memset(ones[:], 1.0)
            ps = psum.tile([2, 1], F32)
            nc.tensor.matmul(out=ps[:], lhsT=s2[:], rhs=ones[:], start=True, stop=True)
            res = pool.tile([2, 1], F32)
            nc.vector.tensor_copy(out=res[:], in_=ps[:])
            nc.sync.dma_start(out=out_ext[:], in_=res[:])

    _preload_act_table(nc)
    return nc


def _make_in_maps(scores, labels, dens, tstar):
    in_maps = []
    for c in range(NCORES):
        r0, r1 = c * BL, (c + 1) * BL
        meta = np.concatenate(
            [
                np.ascontiguousarray(tstar[r0:r1]).reshape(P, J),
                np.ascontiguousarray(dens[r0:r1]).reshape(P, J).view(np.int32),
            ],
            axis=1,
        )
        in_maps.append(
            {
                "scores": np.ascontiguousarray(scores[r0:r1]).reshape(-1),
                "labels": np.ascontiguousarray(labels[r0:r1]).reshape(-1),
                "meta": np.ascontiguousarray(meta),
            }
        )
    return in_maps


def _prep_inputs(inputs):
    scores = np.asarray(inputs["scores"], dtype=np.float32)
    labels = np.asarray(inputs["labels"], dtype=np.float32)
    dens = np.asarray(inputs["checkpoint_density"], dtype=np.float32)
    tstar = np.asarray(inputs["t_star"]).astype(np.int32)
    assert scores.shape == (B, S) and labels.shape == (B, S)
    return _make_in_maps(scores, labels, dens, tstar)


def _combine(per_core_outs):
    parts = np.stack(
        [np.asarray(o, dtype=np.float64).reshape(2) for o in per_core_outs]
    )
    total_loss, n_valid = parts.sum(axis=0)
    if n_valid <= 0:
        return np.zeros((), dtype=np.float32)
    return np.asarray(np.float32(total_loss / max(n_valid, 1.0)))


def get_graph():
    global _graph_cache
    if _graph_cache is None:
        nc = _build_graph()
        # Bacc defers register allocation and multi-wait splitting (HW allows
        # one sync wait per compute instruction) to its compile pass, which
        # runs in finalize().
        nc.finalize()
        _graph_cache = nc
    return _graph_cache


def kernel(**inputs) -> np.ndarray:
    in_maps = _prep_inputs(inputs)
    nc = get_graph()
    res = run_bass_kernel_spmd(nc, in_maps, core_ids=list(range(NCORES))).results
    return _combine([res[i]["out"] for i in range(NCORES)])


# revision 12
# speedup vs baseline: 1.1642x; 1.0514x over previous
"""Trainium2 Bass kernel for AdaptiveWindowLoss (segment_reduce).

Reference semantics (per row b of scores[B,S], labels[B,S]):
    k      = ceil(1 / max(density_b, 0.1))            # k in [1, 10]
    win    = { t : |t - t_star_b| <= k }              # <= 21 columns
    w      = exp(-|t - t_star_b|) * win ; w /= sum(w)
    ref_avg= sum(scores*w*[lab==1 & win]) / max(cnt1, 1)
    dev_avg= sum(scores*w*[lab==0 & win]) / max(cnt0, 1)
    valid  = cnt1>0 and cnt0>0
    loss_b = softplus(-(ref_avg - dev_avg))
    out    = sum(loss_b * valid) / max(n_valid, 1)    (0 if n_valid == 0)

Only the <=21-wide window around t_star matters, so each core gathers a
fixed 21-wide slab per row with one indirect DMA per tensor (per-row
start = clamp(t_star-10, 0, S-21)) instead of reading the full 4096
columns (~0.2% of the naive memory traffic).

Distribution: batch rows sharded 1024/core across 8 cores. Each core
emits [sum(loss*valid), sum(valid)]; the host adds the 8 pairs and does
the final division (16 floats - cheaper than a device AllReduce).

Numerical notes:
 - window mask dist<=ceil(1/d') is evaluated as dist*d' < 1+d' (exact
   in real arithmetic for integer dist; float edge cases are measure-
   zero and bounded by the 2e-2 rel-err gate).
 - masked weights are built inside the Exp argument (60*wm - 60 - dist)
   so no DVE multiply is needed on the ACT output; outside-window
   contributions are <= e^-60.
 - label masks: (lab==0) sums are derived as win_total - (lab==1) sums
   since labels are exactly {0,1}.
"""

import sys

for _p in ("/opt/trn_rl_repo", "/root/.axon_site/_ro/trn_rl_repo"):
    if _p not in sys.path:
        sys.path.append(_p)

import numpy as np

from concourse import bass, bacc, mybir
import concourse.tile as tile
from concourse.bass_utils import run_bass_kernel_spmd
from concourse.hw_specs import get_activation_tables

B, S = 8192, 4096
NCORES = 8
BL = B // NCORES        # 1024 rows per core
P = 128                 # SBUF partitions
J = BL // P             # 8 windows per partition
KMAX = 10               # k = ceil(1/max(d,0.1)) <= 10
W = 2 * KMAX + 1        # 21-wide gather covers every possible window
JW = J * W
F32 = mybir.dt.float32
I32 = mybir.dt.int32

_graph_cache = None


def _preload_act_table(nc):
    """Pre-place one ACT function-table load that covers Exp+Ln so the
    compile pass doesn't insert a second mid-kernel table swap (~2.7us:
    table DMA + forced scalar-engine drain)."""
    tables = get_activation_tables(nc.m.arch)
    need = {
        mybir.ActivationFunctionType.Exp,
        mybir.ActivationFunctionType.Ln,
        mybir.ActivationFunctionType.Identity,
        mybir.ActivationFunctionType.Copy,
    }
    set_id = None
    for i, (_name, funcs) in enumerate(tables.items()):
        if need <= funcs:
            set_id = i
            break
    if set_id is None:
        return  # fall back to automatic placement
    inst = mybir.InstLoadActFuncSet(
        name=nc.get_next_instruction_name(),
        act_func_set_id=set_id,
        ins=[],
        outs=[],
    )
    inst.engine = mybir.EngineType.Activation
    nc.register_instruction(inst)
    entry = nc.main_func.blocks[0]
    pos = 0
    if nc.scalar.preamble_end is not None:
        try:
            pos = entry.instructions.index(nc.scalar.preamble_end) + 1
        except ValueError:
            pos = 0
    entry.instructions.insert(pos, inst)


def _build_graph():
    nc = bacc.Bacc()
    # scores/labels element-interleaved on host: sl[r, t, 0]=scores, [.,.,1]=labels
    sl_ext = nc.declare_dram_parameter("sl", [BL * S * 2], F32, isOutput=False)
    # packed per-row params: cols [0,J) = t_star int32, [J,2J) = density f32 bits
    me_ext = nc.declare_dram_parameter("meta", [P, 2 * J], I32, isOutput=False)
    out_ext = nc.declare_dram_parameter("out", [2, 1], F32, isOutput=True)

    TT = mybir.AluOpType
    AF = mybir.ActivationFunctionType

    with tile.TileContext(nc) as tc:
        with (
            tc.tile_pool(name="sbuf", bufs=1) as pool,
            tc.tile_pool(name="psum", bufs=1, space="PSUM") as psum,
        ):
            def tt(out, in0, in1, op):
                nc.vector.tensor_tensor(out=out, in0=in0, in1=in1, op=op)

            def ts(out, in0, s1, op0, s2=None, op1=None):
                kw = {}
                if op1 is not None:
                    kw = dict(scalar2=s2, op1=op1)
                else:
                    kw = dict(scalar2=None)
                nc.vector.tensor_scalar(out=out, in0=in0, scalar1=s1, op0=op0, **kw)

            def b3(ap2):  # [P, J] -> broadcast [P, J, W]
                return ap2.to_broadcast([P, J, W])

            def v3(ap2):  # [P, n*W] -> [P, n, W] view
                return ap2.rearrange("p (j w) -> p j w", w=W)

            # ---- load packed params early via HWDGE (sync engine); the
            # index math runs on gpsimd so the gather needs no extra
            # cross-engine hop after it.
            meta = pool.tile([P, 2 * J], I32)
            nc.sync.dma_start(out=meta[:], in_=me_ext[:])
            ts_i = meta[:, 0:J]
            dn = meta[:, J : 2 * J].bitcast(F32)

            # rowbase iota in the interleaved array: r = p*J + j -> r*S*2
            rb = pool.tile([P, J], I32)
            nc.gpsimd.iota(
                rb[:], pattern=[[2 * S, J]], base=0, channel_multiplier=J * 2 * S
            )
            # t within window: 0..W-1 repeated J times (exact in f32)
            tor = pool.tile([P, JW], F32)
            nc.gpsimd.iota(
                tor[:], pattern=[[0, J], [1, W]], base=0, channel_multiplier=0,
                allow_small_or_imprecise_dtypes=True,
            )

            # start = clamp(t_star - 10, 0, S-W)   (int32, on gpsimd)
            st_i = pool.tile([P, J], I32)
            nc.gpsimd.tensor_scalar(
                out=st_i[:], in0=ts_i, scalar1=-KMAX, scalar2=0,
                op0=TT.add, op1=TT.max,
            )
            nc.gpsimd.tensor_scalar(
                out=st_i[:], in0=st_i[:], scalar1=S - W, scalar2=None, op0=TT.min
            )
            st2 = pool.tile([P, J], I32)
            nc.gpsimd.tensor_tensor(out=st2[:], in0=st_i[:], in1=st_i[:], op=TT.add)
            idx = pool.tile([P, J], I32)
            nc.gpsimd.tensor_tensor(out=idx[:], in0=rb[:], in1=st2[:], op=TT.add)
            # a = t_star - start = in-window position of t_star
            a_i = pool.tile([P, J], I32)
            nc.gpsimd.tensor_tensor(out=a_i[:], in0=ts_i, in1=st_i[:], op=TT.subtract)

            # ---- ONE indirect gather pulls the interleaved window slab:
            # per row 42 contiguous floats = 21 scores + 21 labels.
            gath = pool.tile([P, J * 2 * W], F32)
            nc.gpsimd.indirect_dma_start(
                out=gath[:], out_offset=None,
                in_=sl_ext[:].rearrange("(a b) -> a b", b=1),
                in_offset=bass.IndirectOffsetOnAxis(ap=idx[:], axis=0),
            )
            gv = gath[:].rearrange("p (j w c) -> p j w c", w=W, c=2)
            scw3 = gv[:, :, :, 0]   # [P, J, W] stride-2 views
            lbw3 = gv[:, :, :, 1]

            # ---- overlapped with the gathers: per-row f32 prep on DVE
            a = pool.tile([P, J], F32)
            nc.vector.tensor_copy(out=a[:], in_=a_i[:])
            dp = pool.tile([P, J], F32)    # d' = max(density, 0.1)
            ts(dp[:], dn, 0.1, TT.max)
            rhs = pool.tile([P, J], F32)   # 1 + d'
            ts(rhs[:], dn, 0.1, TT.max, 1.0, TT.add)

            # dist = |t - a| = max(t-a, a-t)
            d1 = pool.tile([P, JW], F32)
            d2 = pool.tile([P, JW], F32)
            tt(v3(d1[:]), v3(tor[:]), b3(a[:]), TT.subtract)
            tt(v3(d2[:]), b3(a[:]), v3(tor[:]), TT.subtract)
            dist = pool.tile([P, JW], F32)
            tt(dist[:], d1[:], d2[:], TT.max)
            # window mask: dist <= ceil(1/d')  <=>  dist*d' < 1+d'
            lhs = pool.tile([P, JW], F32)
            tt(v3(lhs[:]), v3(dist[:]), b3(dp[:]), TT.mult)

            # big reduce tile: 5 sections of [P, JW]:
            #   0: wn (masked exp weights)   1: wm (window mask)
            #   2: m1 (wm & lab==1)          3: sw (scores * wn)
            #   4: sw1 (sw & lab==1)
            big = pool.tile([P, 5 * JW], F32)
            wm = big[:, 1 * JW : 2 * JW]
            tt(v3(wm), v3(lhs[:]), b3(rhs[:]), TT.is_lt)
            # exp argument: 60*wm - 60 - dist  (= -dist inside the window)
            t1 = pool.tile([P, JW], F32)
            ts(t1[:], wm, 60.0, TT.mult, -60.0, TT.add)
            arg = pool.tile([P, JW], F32)
            tt(arg[:], t1[:], dist[:], TT.subtract)
            wn = big[:, 0:JW]
            nc.scalar.activation(out=wn, in_=arg[:], func=AF.Exp)
            # label mask and products
            l1 = pool.tile([P, JW], F32)
            ts(v3(l1[:]), lbw3, 1.0, TT.is_equal)
            tt(big[:, 2 * JW : 3 * JW], wm, l1[:], TT.mult)       # m1
            sw = big[:, 3 * JW : 4 * JW]
            tt(v3(sw), scw3, v3(wn), TT.mult)                     # sw
            tt(big[:, 4 * JW : 5 * JW], sw, l1[:], TT.mult)       # sw1

            # ---- one reduce for all five sections: [P,5J,W] -> [P,5J]
            red = pool.tile([P, 5 * J], F32)
            nc.vector.tensor_reduce(
                out=red[:], in_=v3(big[:]), axis=mybir.AxisListType.X, op=TT.add
            )
            sum_w = red[:, 0:J]
            cw = red[:, J : 2 * J]          # c1 + c0 (exact)
            c1 = red[:, 2 * J : 3 * J]
            s_tot = red[:, 3 * J : 4 * J]   # s1 + s0 (+ ~1e-25 eps)
            s1 = red[:, 4 * J : 5 * J]

            c0 = pool.tile([P, J], F32)
            tt(c0[:], cw, c1, TT.subtract)
            s0 = pool.tile([P, J], F32)
            tt(s0[:], s_tot, s1, TT.subtract)

            # ---- row-level math on [P, J]
            u1 = pool.tile([P, J], F32)
            u0 = pool.tile([P, J], F32)
            ts(u1[:], c1, 1.0, TT.max)
            ts(u0[:], c0[:], 1.0, TT.max)
            inv_w = pool.tile([P, J], F32)
            inv1 = pool.tile([P, J], F32)
            inv0 = pool.tile([P, J], F32)
            nc.vector.reciprocal(out=inv_w[:], in_=sum_w)
            nc.vector.reciprocal(out=inv1[:], in_=u1[:])
            nc.vector.reciprocal(out=inv0[:], in_=u0[:])
            ra = pool.tile([P, J], F32)
            rd = pool.tile([P, J], F32)
            tt(ra[:], s1, inv1[:], TT.mult)
            tt(rd[:], s0[:], inv0[:], TT.mult)
            delta = pool.tile([P, J], F32)
            tt(delta[:], ra[:], rd[:], TT.subtract)
            tt(delta[:], delta[:], inv_w[:], TT.mult)

            # final reduce tile: [P, 0:J] = loss*valid, [P, J:2J] = valid
            sl2 = pool.tile([P, 2 * J], F32)
            val = sl2[:, J : 2 * J]
            vm = pool.tile([P, J], F32)
            tt(vm[:], c1, c0[:], TT.min)
            ts(val, vm[:], 0.0, TT.is_gt)    # valid = (min(c1,c0) > 0)

            # loss = softplus(-delta) = max(-delta,0) + log(1+exp(-|delta|))
            nneg = pool.tile([P, J], F32)
            ts(nneg[:], delta[:], -1.0, TT.mult)
            mx = pool.tile([P, J], F32)
            ts(mx[:], nneg[:], 0.0, TT.max)
            ad = pool.tile([P, J], F32)
            tt(ad[:], delta[:], nneg[:], TT.max)
            en = pool.tile([P, J], F32)
            nc.scalar.activation(out=en[:], in_=ad[:], func=AF.Exp, scale=-1.0)
            lg = pool.tile([P, J], F32)
            nc.scalar.activation(out=lg[:], in_=en[:], func=AF.Ln, bias=1.0)
            li = pool.tile([P, J], F32)
            tt(li[:], mx[:], lg[:], TT.add)
            tt(sl2[:, 0:J], li[:], val, TT.mult)

            # ---- [P,2,J] -> [P,2] -> matmul with ones -> [2,1] partials
            s2 = pool.tile([P, 2], F32)
            nc.vector.tensor_reduce(
                out=s2[:], in_=sl2[:].rearrange("p (g j) -> p g j", j=J),
                axis=mybir.AxisListType.X, op=TT.add,
            )
            ones = pool.tile([P, 1], F32)
            nc.vector.memset(ones[:], 1.0)
            ps = psum.tile([2, 1], F32)
            nc.tensor.matmul(out=ps[:], lhsT=s2[:], rhs=ones[:], start=True, stop=True)
            res = pool.tile([2, 1], F32)
            nc.vector.tensor_copy(out=res[:], in_=ps[:])
            nc.sync.dma_start(out=out_ext[:], in_=res[:])

    _preload_act_table(nc)
    return nc


def _make_in_maps(scores, labels, dens, tstar):
    # element-interleave scores/labels so one indirect gather fetches both:
    # sl[r, t, 0] = scores[r, t], sl[r, t, 1] = labels[r, t]
    sl = np.empty((B, S, 2), dtype=np.float32)
    sl[:, :, 0] = scores
    sl[:, :, 1] = labels
    in_maps = []
    for c in range(NCORES):
        r0, r1 = c * BL, (c + 1) * BL
        meta = np.concatenate(
            [
                np.ascontiguousarray(tstar[r0:r1]).reshape(P, J),
                np.ascontiguousarray(dens[r0:r1]).reshape(P, J).view(np.int32),
            ],
            axis=1,
        )
        in_maps.append(
            {
                "sl": sl[r0:r1].reshape(-1),
                "meta": np.ascontiguousarray(meta),
            }
        )
    return in_maps


def _prep_inputs(inputs):
    scores = np.asarray(inputs["scores"], dtype=np.float32)
    labels = np.asarray(inputs["labels"], dtype=np.float32)
    dens = np.asarray(inputs["checkpoint_density"], dtype=np.float32)
    tstar = np.asarray(inputs["t_star"]).astype(np.int32)
    assert scores.shape == (B, S) and labels.shape == (B, S)
    return _make_in_maps(scores, labels, dens, tstar)


def _combine(per_core_outs):
    parts = np.stack(
        [np.asarray(o, dtype=np.float64).reshape(2) for o in per_core_outs]
    )
    total_loss, n_valid = parts.sum(axis=0)
    if n_valid <= 0:
        return np.zeros((), dtype=np.float32)
    return np.asarray(np.float32(total_loss / max(n_valid, 1.0)))


def get_graph():
    global _graph_cache
    if _graph_cache is None:
        nc = _build_graph()
        # Bacc defers register allocation and multi-wait splitting (HW allows
        # one sync wait per compute instruction) to its compile pass, which
        # runs in finalize().
        nc.finalize()
        _graph_cache = nc
    return _graph_cache


def kernel(**inputs) -> np.ndarray:
    in_maps = _prep_inputs(inputs)
    nc = get_graph()
    res = run_bass_kernel_spmd(nc, in_maps, core_ids=list(range(NCORES))).results
    return _combine([res[i]["out"] for i in range(NCORES)])
